# revision 1
# baseline (speedup 1.0000x reference)
"""Trainium2 Bass kernel for CoupledClustersLoss.

Reference computation (per class c of 1024; embeddings [65536, 512] f32):
  rows [64c, 64c+64) = 32 "pos" rows then 32 "neg" rows
  anchor = mean(pos)                      [512]
  ap_s   = ||pos_s - anchor||^2           [32]
  an     = min_s ||neg_s - anchor||^2     scalar
  loss_c = sum_s relu(ap_s - an + margin)
  output = mean_c loss_c                  scalar f32

Sharding: 8 cores, each takes 128 consecutive classes (8192 rows, 16 MiB).

Device algorithm (per core), memory-bound design (~56us vs 47us DMA
roofline, measured via repetition-slope on HW):
  - 64 tiles of [128 rows, 512] (2 classes per tile, contiguous in DRAM),
    streamed in 512 KiB group DMAs on the SP HWDGE ring (no gaps).
  - diff = WM.T @ X on TensorE in float32r (1 cyc/row; HW rel-err 9e-7),
    where WM = I - W and W[k,m] = 1/32 iff k is a pos row of m's class.
    One constant [128,128] stationary weight; result is
    (x - anchor_class(x)) for every row, straight into PSUM.
  - Pair-fused squares: 2 matmuls fill one [128,1024] PSUM tile; one ACT
    Square into SBUF scrap; one DVE 3D tensor_reduce writes 2 stats
    columns. ACT ~33us and DVE ~36us both stay under the DMA stream.
    The final pair runs ACT-accum singles to shorten the drain chain.
  - Tail (split in halves; half 0 overlaps the stream): PE-transpose
    stats -> [32 tiles, 128 lanes]; VectorE min over the neg lanes;
    ScalarE relu(ap + (margin - an)) with per-partition bias and
    accum_out -> per-class losses [64, 2], DMA'd out on the ACT ring.
  - Host: sum the 8x[64,2] per-class losses, divide by 1024.
"""

import numpy as np

MARGIN = 0.3
N_CLASSES = 1024
N_SAMPLES = 32
D = 512
N_CORES = 8
ROWS_PER_CORE = 2 * N_CLASSES * N_SAMPLES // N_CORES  # 8192
N_TILES = ROWS_PER_CORE // 128  # 64
TILES_PER_GROUP = 2
N_GROUPS = N_TILES // TILES_PER_GROUP


def _set_group(tpg):
    global TILES_PER_GROUP, N_GROUPS
    TILES_PER_GROUP = tpg
    N_GROUPS = N_TILES // tpg

TRACE = False  # set True (before first kernel() call) to profile; see LAST_RESULTS
LAST_RESULTS = None

# float32r runs the PE at 1 cycle/row instead of fp32's 4 (relaxed-precision
# matmul). Verified on HW: see test.py rel-err. Flip off to fall back.
USE_F32R = True
XPOOL_BUFS = 8
PDIFF_BUFS = 3

_compiled = None


def _weight_matrix() -> np.ndarray:
    wm = np.eye(128, dtype=np.float32)
    for c in (0, 1):
        wm[64 * c : 64 * c + 32, 64 * c : 64 * c + 64] -= np.float32(1.0 / 32.0)
    return wm


def _legalize_multiwaits(nc):
    """Walrus codegen only allows one sync-wait on compute instructions
    (EventSemaphore allows two). Hoist excess waits into standalone
    EventSemaphore instructions on the same engine, placed just before."""
    import concourse.mybir as mybir

    skip = (mybir.InstEventSemaphore,)
    n_fix = 0
    for fn in nc.m.functions:
        for blk in fn.blocks:
            new_insts = []
            for inst in blk.instructions:
                si = inst.sync_info
                if (
                    si is not None
                    and len(si.on_wait) > 1
                    and not isinstance(inst, skip)
                ):
                    waits = list(si.on_wait)
                    keep, extra = waits[0], waits[1:]
                    while extra:
                        chunk, extra = extra[:2], extra[2:]
                        evt = mybir.InstEventSemaphore(
                            name=f"evtw-{nc.next_id()}", ins=[], outs=[]
                        )
                        evt.engine = inst.engine
                        evt.sync_info = mybir.SyncInfo(
                            on_wait=chunk, on_update=[]
                        )
                        new_insts.append(evt)
                    inst.sync_info = mybir.SyncInfo(
                        on_wait=[keep], on_update=list(si.on_update)
                    )
                    n_fix += 1
                new_insts.append(inst)
            if len(new_insts) != len(blk.instructions):
                blk.instructions = new_insts
    return n_fix


def _hoist_first_dma(nc):
    """The first embedding DMA has no waits, but sits behind the ~1.2us Bass
    init preamble (register moves + drain + all-engine barrier) on the SP
    queue. Move it into the preamble block before SP's drain so the HBM
    stream starts immediately; everything downstream is pinned to its end."""
    import concourse.mybir as mybir

    SP = mybir.EngineType.SP
    blocks = [b for fn in nc.m.functions for b in fn.blocks]
    if len(blocks) < 2:
        return False
    first = blocks[0]
    # insertion point: before SP's first Drain in the preamble block
    ins_at = None
    for i, inst in enumerate(first.instructions):
        if inst.engine == SP and isinstance(inst, mybir.InstDrain):
            ins_at = i
            break
    if ins_at is None:
        return False
    for blk in blocks[1:]:
        for j, inst in enumerate(blk.instructions):
            if inst.engine == SP and isinstance(inst, mybir.InstDMACopy):
                si = inst.sync_info
                if si is not None and si.on_wait:
                    return False  # first SP DMA has waits; don't touch
                insts = list(blk.instructions)
                insts.pop(j)
                blk.instructions = insts
                finsts = list(first.instructions)
                finsts.insert(ins_at, inst)
                first.instructions = finsts
                return True
    return False


def _build(reps: int = 1):
    from contextlib import ExitStack

    import concourse.bass as bass
    import concourse.mybir as mybir
    import concourse.tile as tile

    f32 = mybir.dt.float32
    AF = mybir.ActivationFunctionType
    Alu = mybir.AluOpType

    # float32r (relaxed-precision matmul input) runs PE at 1 cycle/row vs
    # fp32's 4. The BIR verifier requires f32r matmul inputs to be produced
    # as f32r, so emb/wm are declared f32r end to end (same 4-byte values;
    # the DMA just propagates the dtype).
    fmm = mybir.dt.float32r if USE_F32R else f32
    nc = bass.Bass()
    emb = nc.declare_dram_parameter("emb", [ROWS_PER_CORE, D], fmm, isOutput=False)
    wm_d = nc.declare_dram_parameter("wm", [128, 128], fmm, isOutput=False)
    # Width padded by (reps-1): gives each bench rep-variant a distinct
    # executable signature (the PJRT-side cache otherwise aliases them).
    id_d = nc.declare_dram_parameter(
        "ident", [128, 128 + reps - 1], f32, isOutput=False
    )
    out_d = nc.declare_dram_parameter("out", [64, 2], f32, isOutput=True)

    with tile.TileContext(nc) as tc, ExitStack() as ctx:
        const_pool = ctx.enter_context(tc.tile_pool(name="const", bufs=1))
        xpool = ctx.enter_context(tc.tile_pool(name="xp", bufs=XPOOL_BUFS))
        pdiff = ctx.enter_context(
            tc.tile_pool(name="pdiff", bufs=PDIFF_BUFS, space="PSUM")
        )
        pepi = ctx.enter_context(tc.tile_pool(name="pepi", bufs=1, space="PSUM"))
        spool = ctx.enter_context(tc.tile_pool(name="sp", bufs=1))

        sqpool = ctx.enter_context(tc.tile_pool(name="sq", bufs=3))

        # Const loads are issued on the SP ring AFTER the first embedding
        # group's DMA (see the g==0 hook below) so the stream starts at the
        # earliest possible point; the first matmul needs wm only ~1.5us in.
        wm_sb = const_pool.tile([128, 128], fmm, tag="wm", name="wm_sb")
        id_sb = const_pool.tile([128, 128], f32, tag="ident", name="id_sb")

        def load_consts():
            nc.sync.dma_start(wm_sb[:], wm_d[:])
            nc.sync.dma_start(id_sb[:], id_d[:, 0:128])

        stats = spool.tile([128, N_TILES], f32, tag="stats", name="stats")

        # The fused 4-byte matmul (internal LDWEIGHTS) only supports a single
        # sync-wait in walrus codegen. Tiny "gate" matmuls absorb each DMA
        # wait on PE so real matmuls carry at most one wait (PSUM release).
        # Any other excess waits are hoisted by _legalize_multiwaits.
        gate_ps = pepi.tile([1, 1], f32, tag="gate", name="gate_ps")

        def pe_gate(ap):
            # f32 view: f32r has ISA restrictions on tiny free dims, and the
            # gate's only job is to absorb a DMA wait on the PE queue.
            if ap.dtype == mybir.dt.float32r:
                ap = ap.bitcast(f32)
            nc.tensor.matmul(gate_ps[:], lhsT=ap, rhs=ap)

        # emb rows (g b p) d: group g, tile-in-group b, partition p
        emb_r = emb[:].rearrange(
            "(g b p) d -> g p b d", g=N_GROUPS, b=TILES_PER_GROUP, p=128
        )
        # Pair-fused hot loop: 2 matmuls fill one [128, 2*D] PSUM tile
        # (2 banks), one ACT Square (no accum) squares the pair into an SBUF
        # scrap, one DVE 3D tensor_reduce produces both stats columns.
        # Engine budgets/tile-pair: PE 2x~215ns, ACT ~870ns, DVE ~1100ns —
        # all under the ~1460ns/pair DMA streaming rate.
        assert TILES_PER_GROUP % 2 == 0 or TILES_PER_GROUP == 1

        def tail_half(r, h, loss):
            """Per-class losses for stats columns [h*32, (h+1)*32). Half 0
            runs mid-stream (overlapped); half 1 is the only end-of-kernel
            tail. Out-DMA rides the ACT HWDGE ring so the SP ring keeps
            streaming embeddings."""
            c0 = h * 32
            statsT = pepi.tile([32, 128], f32, tag="statsT", name=f"sT{r}_{h}")
            nc.tensor.transpose(statsT[:], stats[:, c0 : c0 + 32], id_sb[:])
            anmin = spool.tile([32, 2], f32, tag="anmin", name=f"am{r}_{h}")
            nc.vector.tensor_reduce(
                anmin[:, 0:1], statsT[:, 32:64], axis=mybir.AxisListType.X, op=Alu.min
            )
            nc.vector.tensor_reduce(
                anmin[:, 1:2], statsT[:, 96:128], axis=mybir.AxisListType.X, op=Alu.min
            )
            # bias = margin - an  (= an * -1 + margin, immediates on DVE)
            biasv = spool.tile([32, 2], f32, tag="biasv", name=f"bv{r}_{h}")
            nc.vector.tensor_scalar(biasv[:], anmin[:], -1.0, MARGIN, Alu.mult, Alu.add)
            # Absorb the PE (statsT) dependency on ACT with a dummy copy so
            # the relu activations carry only the DVE (biasv) wait.
            tg = spool.tile([1, 1], f32, tag="tail_gate", name=f"tg{r}_{h}")
            nc.scalar.activation(tg[:], statsT[0:1, 0:1], AF.Copy)
            junkA = spool.tile([32, 32], f32, tag="junk", name=f"jA{r}_{h}")
            nc.scalar.activation(
                junkA[:],
                statsT[:, 0:32],
                AF.Relu,
                bias=biasv[:, 0:1],
                accum_out=loss[c0 : c0 + 32, 0:1],
            )
            junkB = spool.tile([32, 32], f32, tag="junk", name=f"jB{r}_{h}")
            nc.scalar.activation(
                junkB[:],
                statsT[:, 64:96],
                AF.Relu,
                bias=biasv[:, 1:2],
                accum_out=loss[c0 : c0 + 32, 1:2],
            )
            nc.scalar.dma_start(out_d[c0 : c0 + 32, :], loss[c0 : c0 + 32, :])

        for r in range(reps):
            loss = spool.tile([64, 2], f32, tag="loss", name=f"loss{r}")
            for g in range(N_GROUPS):
                xg = xpool.tile(
                    [128, TILES_PER_GROUP * D], fmm, tag="xg", name=f"xg{r}_{g}"
                )
                nc.sync.dma_start(
                    xg[:].rearrange("p (b d) -> p b d", b=TILES_PER_GROUP), emb_r[g]
                )
                if r == 0 and g == 0:
                    load_consts()
                    pe_gate(wm_sb[:, 0:1])
                    pe_gate(id_sb[:, 0:1])
                pe_gate(xg[:, 0:1])
                last_group = g == N_GROUPS - 1
                for b in range(0, TILES_PER_GROUP, 2):
                    t = g * TILES_PER_GROUP + b
                    dpair = pdiff.tile(
                        [128, 2 * D], f32, tag="dpair", name=f"dp{r}_{t}"
                    )
                    nc.tensor.matmul(
                        dpair[:, 0:D], lhsT=wm_sb[:], rhs=xg[:, b * D : (b + 1) * D]
                    )
                    nc.tensor.matmul(
                        dpair[:, D : 2 * D],
                        lhsT=wm_sb[:],
                        rhs=xg[:, (b + 1) * D : (b + 2) * D],
                    )
                    if last_group and b + 2 == TILES_PER_GROUP:
                        # Final pair: ACT-only square+accum singles, dropping
                        # the DVE reduce from the end-of-kernel drain chain.
                        for i in (0, 1):
                            sqh = sqpool.tile(
                                [128, D], f32, tag="sqh", bufs=1, name=f"sqh{r}_{i}"
                            )
                            nc.scalar.activation(
                                sqh[:],
                                dpair[:, i * D : (i + 1) * D],
                                AF.Square,
                                accum_out=stats[:, t + i : t + i + 1],
                            )
                    else:
                        sqp = sqpool.tile(
                            [128, 2 * D], f32, tag="sqp", bufs=3, name=f"sqp{r}_{t}"
                        )
                        nc.scalar.activation(sqp[:], dpair[:], AF.Square)
                        nc.vector.tensor_reduce(
                            stats[:, t : t + 2],
                            sqp[:].rearrange("p (b d) -> p b d", b=2),
                            axis=mybir.AxisListType.X,
                            op=Alu.add,
                        )
                if (g + 1) * TILES_PER_GROUP == 32:
                    tail_half(r, 0, loss)
            tail_half(r, 1, loss)

    _legalize_multiwaits(nc)
    # NOTE: hoisting the first DMA before the init barrier measured ~0.5us
    # faster but intermittently wedged the device (NRT_EXEC_UNIT_UNRECOVERABLE)
    # - deliberately NOT enabled.
    return nc


def kernel(embeddings: np.ndarray, target: np.ndarray) -> np.ndarray:
    global _compiled, LAST_RESULTS
    from concourse.bass_utils import run_bass_kernel_spmd

    if _compiled is None:
        _compiled = _build()
    nc = _compiled

    emb = np.ascontiguousarray(np.asarray(embeddings, dtype=np.float32))
    shards = emb.reshape(N_CORES, ROWS_PER_CORE, D)
    wm = _weight_matrix()
    ident = np.eye(128, dtype=np.float32)
    in_maps = [
        {"emb": shards[i], "wm": wm, "ident": ident} for i in range(N_CORES)
    ]
    res = run_bass_kernel_spmd(
        nc, in_maps, core_ids=list(range(N_CORES)), trace=TRACE
    )
    LAST_RESULTS = res
    losses = np.stack([res.results[i]["out"] for i in range(N_CORES)])  # [8, 64, 2]
    total = losses.astype(np.float64).sum() / N_CLASSES
    return np.float32(total)

